# revision 27
# baseline (speedup 1.0000x reference)
"""Trainium2 Bass kernel for nn_AttentionCross (dual-direction masked cross attention).

Computation per batch b (reference semantics):
    v   = videofea.T                      [T, vd]
    q   = split_heads(textfea @ Wq + bq)  [g, L, d]
    k   = split_heads(v @ Wk + bk)        [g, T, d]
    vv  = split_heads(textfea @ Wvv+bvv)  [g, L, d]
    vt  = split_heads(v @ Wvt + bvt)      [g, T, d]
    att = q @ k.T (masked; -1e9 where mask==0)        [g, L, T]
    att_t = softmax_T(att)/32 ; att_v = softmax_L(att.T)/32
    out_v = att_v @ vv   -> [b, g*d, T]
    out_t = att_t @ vt   -> [b, L, g*d]

Strategy: data-parallel over batch across 8 NeuronCores (4 batches/core).

Numerics: projections run in bf16 except the q path (f32) and the S matmuls
(f32 with f32-stored q,k) — rounding of the exp() argument is the dominant
error term, so k is produced in f32 (spilled to DRAM; too big for SBUF) and
q in f32. Softmax uses a fixed offset C (exact after normalization; value
range is bounded). The 1/sqrt(dim) output scale is pre-folded into vt/vv.

The attention matrix is materialized in both layouts by two matmul passes
(S in [L,T] for out_v's operand; S^T in [T,L] tiles for out_t's T-contraction)
— each pass exp'd and masked in its own layout. All softmax reductions are
free-dim reductions (on GpSimd, which is otherwise idle); normalizations are
per-output-partition scales except out_v's per-column one, which is applied
via a partition-broadcast reciprocal tile at the PSUM->SBUF copy (off the PE
critical path). out_v is computed as out_v^T = vv^T @ P (one weight load,
N=512 moving operand) and lands in the reference's [b, g*d, T] layout with
fully contiguous DMA. The group loop is software-pipelined with skew 1 so
the PE never waits on the exp/mask lag of the current group.

bvt is folded in post-hoc: sum_t att_t[l,t] == 1/32 exactly, so
out_t += bvt/32 after the matmul; bvv is added into vv directly.
"""

import sys

if "/opt/trn_rl_repo" not in sys.path:
    sys.path.insert(0, "/opt/trn_rl_repo")

import numpy as np

import concourse.bass as bass
import concourse.mybir as mybir
import concourse.tile as tile
from concourse import bacc
from concourse.alu_op_type import AluOpType
from concourse.bass_utils import run_bass_kernel_spmd
from concourse.masks import make_identity

F32 = mybir.dt.float32
BF16 = mybir.dt.bfloat16
I32 = mybir.dt.int32
AF = mybir.ActivationFunctionType

# Problem constants
B, VD, T_FULL, TD, L, A, G = 32, 1024, 2048, 768, 128, 1024, 8
D = A // G  # 128
N_CORES = 8
B_PC = B // N_CORES  # 4 batches per core
SCALE = 32.0
EXP_C = 24.0  # fixed softmax offset; |att| << 24 for this data distribution


def build_kernel(b_pc: int = B_PC, t: int = T_FULL):
    """Build the per-core Bass program. Returns the compiled Bacc object."""
    nc = bacc.Bacc("TRN2", target_bir_lowering=False, debug=False)

    assert t % 1024 == 0
    nt = t // 128  # T tiles
    nch = t // 512  # 512-wide chunks
    npair = t // 1024  # 1024-wide chunk pairs
    kv = VD // 128  # 8 k-tiles over video dim
    kt = TD // 128  # 6 k-tiles over text dim

    videofea = nc.dram_tensor("videofea", [b_pc, VD, t], F32, kind="ExternalInput").ap()
    textfea = nc.dram_tensor("textfea", [b_pc, L, TD], F32, kind="ExternalInput").ap()
    mask = nc.dram_tensor("mask", [b_pc, t, L], I32, kind="ExternalInput").ap()
    wq = nc.dram_tensor("Wq", [TD, A], F32, kind="ExternalInput").ap()
    bq = nc.dram_tensor("bq", [A], F32, kind="ExternalInput").ap()
    wk = nc.dram_tensor("Wk", [VD, A], F32, kind="ExternalInput").ap()
    bk = nc.dram_tensor("bk", [A], F32, kind="ExternalInput").ap()
    wvv = nc.dram_tensor("Wvv", [TD, A], F32, kind="ExternalInput").ap()
    bvv = nc.dram_tensor("bvv", [A], F32, kind="ExternalInput").ap()
    wvt = nc.dram_tensor("Wvt", [VD, A], F32, kind="ExternalInput").ap()
    bvt = nc.dram_tensor("bvt", [A], F32, kind="ExternalInput").ap()

    # out_v in the reference's [b, g*d, T] layout directly
    out_v = nc.dram_tensor("out_v", [b_pc, A, t], F32, kind="ExternalOutput").ap()
    out_t = nc.dram_tensor("out_t", [b_pc, L, A], F32, kind="ExternalOutput").ap()

    # internal DRAM scratch
    m_scr = nc.dram_tensor("m_scr", [b_pc, t, L], BF16).ap()  # mask transpose bounce
    kt_scr = nc.dram_tensor("kt_scr", [b_pc, G, 128, t], F32).ap()  # f32 kT spill
    rv_scr = nc.dram_tensor("rv_scr", [G, t], F32).ap()  # colsum recip bounce

    def bcast_ap(vec: bass.AP, parts: int = 128) -> bass.AP:
        # [N] dram vector -> [parts, N] partition-broadcast AP (for DMA)
        return bass.AP(tensor=vec.tensor, offset=vec.offset, ap=[[0, parts]] + list(vec.ap))

    from contextlib import ExitStack

    with tile.TileContext(nc) as tc:
        with ExitStack() as ctx:
            ec = ctx.enter_context
            consts = ec(tc.tile_pool(name="consts", bufs=1))
            vid_pool = ec(tc.tile_pool(name="vid", bufs=1))
            txt_pool = ec(tc.tile_pool(name="txt", bufs=1))
            wqs_pool = ec(tc.tile_pool(name="wqs", bufs=2))
            vt_pool = ec(tc.tile_pool(name="vt", bufs=1))
            ktg_pool = ec(tc.tile_pool(name="ktg", bufs=2))
            masktl_pool = ec(tc.tile_pool(name="masktl", bufs=1))
            masklt_pool = ec(tc.tile_pool(name="masklt", bufs=1))
            qv_pool = ec(tc.tile_pool(name="qv", bufs=1))
            p_pool = ec(tc.tile_pool(name="p", bufs=2))
            pt_pool = ec(tc.tile_pool(name="pt", bufs=2))
            wide_pool = ec(tc.tile_pool(name="wide", bufs=2))  # kT stage
            rvb_pool = ec(tc.tile_pool(name="rvb", bufs=1))
            small_pool = ec(tc.tile_pool(name="small", bufs=4))
            ostage_pool = ec(tc.tile_pool(name="ostage", bufs=2))
            ov_pool = ec(tc.tile_pool(name="ovstage", bufs=1))
            ps_bank = ec(tc.tile_pool(name="ps_bank", bufs=2, space="PSUM"))  # 2-bank
            ps_acc = ec(tc.tile_pool(name="ps_acc", bufs=4, space="PSUM"))  # 1-bank

            # ---------------- constants ----------------
            wvv_sb = consts.tile([128, kt, A], BF16)
            nc.gpsimd.dma_start(out=wvv_sb, in_=wvv.rearrange("(k p) a -> p k a", p=128))
            wk_sb = consts.tile([128, kv, A], BF16)
            nc.gpsimd.dma_start(out=wk_sb, in_=wk.rearrange("(k p) a -> p k a", p=128))
            wvt_sb = consts.tile([128, kv, A], BF16)
            nc.gpsimd.dma_start(out=wvt_sb, in_=wvt.rearrange("(k p) a -> p k a", p=128))

            bk_sb = consts.tile([128, G], F32)
            nc.sync.dma_start(out=bk_sb, in_=bk.rearrange("(g p) -> p g", p=128))
            # bq as a single-partition row for the K=1 bias matmul
            bq_row = consts.tile([1, A], F32)
            nc.sync.dma_start(out=bq_row, in_=bq[None, :])
            ones_row = consts.tile([1, L], F32)
            nc.vector.memset(ones_row, 1.0)

            # bvt/SCALE and bvv/SCALE broadcast along partitions
            bvt_sb = consts.tile([128, A], BF16)
            nc.gpsimd.dma_start(out=bvt_sb, in_=bcast_ap(bvt))
            nc.vector.tensor_scalar_mul(bvt_sb, bvt_sb, 1.0 / SCALE)
            bvv_sb = consts.tile([128, A], BF16)
            nc.gpsimd.dma_start(out=bvv_sb, in_=bcast_ap(bvv))
            nc.vector.tensor_scalar_mul(bvv_sb, bvv_sb, 1.0 / SCALE)

            identity = consts.tile([128, 128], F32)
            make_identity(nc, identity)

            neg_c = consts.tile([128, 1], F32)
            nc.vector.memset(neg_c, -EXP_C)

            for bi in range(b_pc):
                # ---------------- per-batch loads ----------------
                video_sb = vid_pool.tile([128, kv, t], BF16, tag="video")
                nc.gpsimd.dma_start(
                    out=video_sb, in_=videofea[bi].rearrange("(k p) t -> p k t", p=128)
                )
                text_sb = txt_pool.tile([128, TD], F32, tag="text")
                nc.sync.dma_start(out=text_sb, in_=textfea[bi])

                # mask: int32 [T, L] --cast--> bf16, bounce through DRAM for the
                # xbar transpose into [L, T]
                m_tl = masktl_pool.tile([128, nt, L], BF16, tag="mtl")
                nc.gpsimd.dma_start(
                    out=m_tl, in_=mask[bi].rearrange("(tt p) l -> p tt l", p=128)
                )
                nc.sync.dma_start(
                    out=m_scr[bi].rearrange("(tt p) l -> p tt l", p=128), in_=m_tl
                )
                m_lt = masklt_pool.tile([L, t], BF16, tag="mlt")
                nc.sync.dma_start_transpose(out=m_lt, in_=m_scr[bi])

                # textfea^T via PE transpose: f32 feeds q, bf16 feeds vv
                textT_sb = txt_pool.tile([128, kt, L], F32, tag="textT")
                textT_bf = txt_pool.tile([128, kt, L], BF16, tag="textT_bf")
                for k in range(kt):
                    ps_tr = ps_acc.tile([128, 128], F32, tag="acc")
                    nc.tensor.transpose(ps_tr, text_sb[:, bass.ts(k, 128)], identity)
                    nc.vector.tensor_copy(out=textT_sb[:, k], in_=ps_tr)
                    nc.vector.tensor_copy(out=textT_bf[:, k], in_=ps_tr)

                # ---------------- projections ----------------
                # qT [A(g), L] f32; Wq streamed from DRAM per k-tile (k-outer);
                # one [128, 8*L] psum tile held across the k loop; bq added via
                # a K=1 matmul against a ones row. Only the very first matmul
                # sets start (it clears the whole bank's has_written bits);
                # later first-touches overwrite per element.
                ps_q = ps_bank.tile([128, G, L], F32, tag="bank", name="ps_q")
                nc.vector.memset(ps_q, 0.0)
                for k in range(kt):
                    wq_k = wqs_pool.tile([128, A], F32, tag="wq")
                    nc.sync.dma_start(out=wq_k, in_=wq[bass.ts(k, 128), :])
                    for g in range(G):
                        nc.tensor.matmul(
                            ps_q[:, g],
                            lhsT=wq_k[:, bass.ts(g, 128)],
                            rhs=textT_sb[:, k],
                            start=False,
                            stop=False,
                            skip_group_check=True,
                        )
                qT_sb = qv_pool.tile([128, G, L], F32, tag="qT")
                for g in range(G):
                    nc.tensor.matmul(
                        ps_q[:, g],
                        lhsT=bq_row[:, bass.ts(g, 128)],
                        rhs=ones_row,
                        start=False,
                        stop=(g == G - 1),
                        skip_group_check=True,
                    )
                nc.vector.tensor_copy(out=qT_sb, in_=ps_q)

                # vv [L, A] bf16, pre-scaled by 1/32, bvv/32 added
                vv_sb = qv_pool.tile([128, A], BF16, tag="vv")
                for half in range(2):
                    ps = ps_acc.tile([128, 512], F32, tag="acc")
                    for k in range(kt):
                        nc.tensor.matmul(
                            ps,
                            lhsT=textT_bf[:, k],
                            rhs=wvv_sb[:, k, bass.ts(half, 512)],
                            start=(k == 0),
                            stop=(k == kt - 1),
                        )
                    nc.vector.scalar_tensor_tensor(
                        out=vv_sb[:, bass.ts(half, 512)],
                        in0=ps,
                        scalar=1.0 / SCALE,
                        op0=AluOpType.mult,
                        in1=bvv_sb[:, bass.ts(half, 512)],
                        op1=AluOpType.add,
                    )

                # vt [T, A] bf16 resident, pre-scaled by 1/32
                vt_sb = vt_pool.tile([128, nt, A], BF16, tag="vt")
                for mt in range(nt):
                    ps = ps_bank.tile([128, A], F32, tag="bank")
                    for k in range(kv):
                        for half in range(2):
                            nc.tensor.matmul(
                                ps[:, bass.ts(half, 512)],
                                lhsT=video_sb[:, k, bass.ts(mt, 128)],
                                rhs=wvt_sb[:, k, bass.ts(half, 512)],
                                start=(k == 0),
                                stop=(k == kv - 1),
                            )
                    nc.vector.tensor_scalar(
                        out=vt_sb[:, mt], in0=ps, scalar1=1.0 / SCALE,
                        scalar2=None, op0=AluOpType.mult,
                    )

                # kT [A(g), T] f32 with bias; spilled to DRAM, re-read per group
                for g in range(G):
                    for pair in range(npair):
                        ps = ps_bank.tile([128, 1024], F32, tag="bank")
                        for k in range(kv):
                            for ch in range(2):
                                nc.tensor.matmul(
                                    ps[:, bass.ts(ch, 512)],
                                    lhsT=wk_sb[:, k, bass.ts(g, 128)],
                                    rhs=video_sb[:, k, bass.ds(pair * 1024 + ch * 512, 512)],
                                    start=(k == 0),
                                    stop=(k == kv - 1),
                                )
                        k_stage = wide_pool.tile([128, 1024], F32, tag="wide")
                        nc.vector.tensor_scalar(
                            out=k_stage, in0=ps, scalar1=bk_sb[:, g : g + 1],
                            scalar2=None, op0=AluOpType.add,
                        )
                        nc.sync.dma_start(
                            out=kt_scr[bi, g, :, bass.ts(pair, 1024)], in_=k_stage
                        )

                # -------- per-group attention (skew-1 software pipeline) -----
                state = {}

                def phase_a(g):
                    st = {}
                    kT_g = ktg_pool.tile([128, t], F32, tag="kT_g", name="kT_g")
                    nc.sync.dma_start(out=kT_g, in_=kt_scr[bi, g])

                    # S [L, T] f32; exp chunks; one wide mask-mult
                    p_sb = p_pool.tile([L, t], BF16, tag="P", name="p_sb")
                    for ch in range(nch):
                        ps = ps_acc.tile([128, 512], F32, tag="acc", name="ps_s")
                        nc.tensor.matmul(
                            ps, lhsT=qT_sb[:, g], rhs=kT_g[:, bass.ts(ch, 512)],
                            start=True, stop=True,
                        )
                        nc.scalar.activation(
                            out=p_sb[:, bass.ts(ch, 512)], in_=ps, func=AF.Exp,
                            bias=neg_c,
                        )
                    nc.vector.tensor_tensor(
                        out=p_sb, in0=p_sb, in1=m_lt, op=AluOpType.mult
                    )
                    rs = small_pool.tile([128, 1], F32, tag="rs", name="rs")
                    nc.vector.tensor_reduce(
                        out=rs, in_=p_sb, axis=mybir.AxisListType.X, op=AluOpType.add
                    )
                    recip_t = small_pool.tile([128, 1], F32, tag="recip_t", name="rt")
                    nc.vector.reciprocal(out=recip_t, in_=rs)

                    # S^T [T, L] tiles; exp + mask per 4-tile chunk; colsums
                    pt_sb = pt_pool.tile([128, nt, L], BF16, tag="PT", name="pt_sb")
                    cs = small_pool.tile([128, nt], F32, tag="cs", name="cs")
                    for grp in range(nch):
                        ps = ps_acc.tile([128, 4, 128], F32, tag="acc", name="ps_st")
                        for j in range(4):
                            tt = grp * 4 + j
                            nc.tensor.matmul(
                                ps[:, j],
                                lhsT=kT_g[:, bass.ts(tt, 128)],
                                rhs=qT_sb[:, g],
                                start=(j == 0),
                                stop=(j == 3),
                                skip_group_check=True,
                            )
                        sl = slice(grp * 4, grp * 4 + 4)
                        nc.scalar.activation(
                            out=pt_sb[:, sl], in_=ps, func=AF.Exp, bias=neg_c
                        )
                        nc.vector.tensor_tensor(
                            out=pt_sb[:, sl], in0=pt_sb[:, sl], in1=m_tl[:, sl],
                            op=AluOpType.mult,
                        )
                        nc.vector.tensor_reduce(
                            out=cs[:, sl], in_=pt_sb[:, sl],
                            axis=mybir.AxisListType.X, op=AluOpType.add,
                        )
                    rv = small_pool.tile([128, nt], F32, tag="rv", name="rv")
                    nc.vector.reciprocal(out=rv, in_=cs)
                    nc.sync.dma_start(
                        out=rv_scr[g].rearrange("(tt p) -> p tt", p=128), in_=rv
                    )
                    rvb = rvb_pool.tile([128, t], F32, tag="rvb", name="rvb")
                    nc.sync.dma_start(out=rvb, in_=bcast_ap(rv_scr[g]))

                    st["p_sb"], st["pt_sb"] = p_sb, pt_sb
                    st["recip_t"], st["rvb"] = recip_t, rvb
                    state[g] = st

                def phase_b(g):
                    st = state.pop(g)
                    p_sb, pt_sb = st["p_sb"], st["pt_sb"]

                    # out_t[L, d_g] = (PT.T @ vt_g) * recip_t + bvt_g/32
                    ps_ot = ps_acc.tile([128, 128], F32, tag="acc", name="ps_ot")
                    for tt in range(nt):
                        nc.tensor.matmul(
                            ps_ot,
                            lhsT=pt_sb[:, tt],
                            rhs=vt_sb[:, tt, bass.ts(g, 128)],
                            start=(tt == 0),
                            stop=(tt == nt - 1),
                        )
                    ot = ostage_pool.tile([128, 128], F32, tag="ot", name="ot")
                    nc.vector.scalar_tensor_tensor(
                        out=ot,
                        in0=ps_ot,
                        scalar=st["recip_t"],
                        op0=AluOpType.mult,
                        in1=bvt_sb[:, bass.ts(g, 128)],
                        op1=AluOpType.add,
                    )
                    nc.sync.dma_start(out=out_t[bi, :, bass.ts(g, 128)], in_=ot)

                    # out_v^T[d_g, T] = vv_g.T @ P ; column normalizer applied
                    # at the PSUM->SBUF copy via the broadcast reciprocal
                    ovT = ov_pool.tile([128, t], F32, tag="ovT", name="ovT")
                    for pair in range(npair):
                        ps = ps_bank.tile([128, 1024], F32, tag="bank", name="ps_ov")
                        for ch in range(2):
                            nc.tensor.matmul(
                                ps[:, bass.ts(ch, 512)],
                                lhsT=vv_sb[:, bass.ts(g, 128)],
                                rhs=p_sb[:, bass.ds(pair * 1024 + ch * 512, 512)],
                                start=True,
                                stop=True,
                            )
                        nc.vector.tensor_tensor(
                            out=ovT[:, bass.ts(pair, 1024)],
                            in0=ps,
                            in1=st["rvb"][:, bass.ts(pair, 1024)],
                            op=AluOpType.mult,
                        )
                    nc.sync.dma_start(out=out_v[bi, bass.ts(g, 128), :], in_=ovT)

                for g in range(G):
                    phase_a(g)
                    if g >= 1:
                        phase_b(g - 1)
                phase_b(G - 1)

    nc.compile()
    return nc


_NC_CACHE: dict = {}


def _get_nc():
    if "nc" not in _NC_CACHE:
        _NC_CACHE["nc"] = build_kernel()
    return _NC_CACHE["nc"]


def kernel(**inputs) -> tuple:
    nc = _get_nc()
    in_maps = []
    for c in range(N_CORES):
        sl = slice(c * B_PC, (c + 1) * B_PC)
        in_maps.append(
            {
                "videofea": np.ascontiguousarray(inputs["videofea"][sl]),
                "textfea": np.ascontiguousarray(inputs["textfea"][sl]),
                "mask": np.ascontiguousarray(inputs["mask"][sl]),
                "Wq": np.asarray(inputs["Wq"]),
                "bq": np.asarray(inputs["bq"]),
                "Wk": np.asarray(inputs["Wk"]),
                "bk": np.asarray(inputs["bk"]),
                "Wvv": np.asarray(inputs["Wvv"]),
                "bvv": np.asarray(inputs["bvv"]),
                "Wvt": np.asarray(inputs["Wvt"]),
                "bvt": np.asarray(inputs["bvt"]),
            }
        )
    res = run_bass_kernel_spmd(nc, in_maps, core_ids=list(range(N_CORES)))
    out_v = np.concatenate([r["out_v"] for r in res.results], axis=0)  # [B, A, T]
    out_t = np.concatenate([r["out_t"] for r in res.results], axis=0)  # [B, L, A]
    return out_v, out_t


# revision 30
# speedup vs baseline: 1.0737x; 1.0737x over previous
"""Trainium2 Bass kernel for nn_AttentionCross (dual-direction masked cross attention).

Computation per batch b (reference semantics):
    v   = videofea.T                      [T, vd]
    q   = split_heads(textfea @ Wq + bq)  [g, L, d]
    k   = split_heads(v @ Wk + bk)        [g, T, d]
    vv  = split_heads(textfea @ Wvv+bvv)  [g, L, d]
    vt  = split_heads(v @ Wvt + bvt)      [g, T, d]
    att = q @ k.T (masked; -1e9 where mask==0)        [g, L, T]
    att_t = softmax_T(att)/32 ; att_v = softmax_L(att.T)/32
    out_v = att_v @ vv   -> [b, g*d, T]
    out_t = att_t @ vt   -> [b, L, g*d]

Strategy: data-parallel over batch across 8 NeuronCores (4 batches/core).

Numerics: projections run in bf16 except the q path (f32) and the S matmuls
(f32 with f32-stored q,k) — rounding of the exp() argument is the dominant
error term, so k is produced in f32 (spilled to DRAM; too big for SBUF) and
q in f32. Softmax uses a fixed offset C (exact after normalization; value
range is bounded). The 1/sqrt(dim) output scale is pre-folded into vt/vv.

The attention matrix is materialized in both layouts by two matmul passes
(S in [L,T] for out_v's operand; S^T in [T,L] tiles for out_t's T-contraction)
— each pass exp'd and masked in its own layout. All softmax reductions are
free-dim reductions (on GpSimd, which is otherwise idle); normalizations are
per-output-partition scales except out_v's per-column one, which is applied
via a partition-broadcast reciprocal tile at the PSUM->SBUF copy (off the PE
critical path). out_v is computed as out_v^T = vv^T @ P (one weight load,
N=512 moving operand) and lands in the reference's [b, g*d, T] layout with
fully contiguous DMA. The group loop is software-pipelined with skew 1 so
the PE never waits on the exp/mask lag of the current group.

bvt is folded in post-hoc: sum_t att_t[l,t] == 1/32 exactly, so
out_t += bvt/32 after the matmul; bvv is added into vv directly.
"""

import sys

if "/opt/trn_rl_repo" not in sys.path:
    sys.path.insert(0, "/opt/trn_rl_repo")

import numpy as np

import concourse.bass as bass
import concourse.mybir as mybir
import concourse.tile as tile
from concourse import bacc
from concourse.alu_op_type import AluOpType
from concourse.bass_utils import run_bass_kernel_spmd
from concourse.masks import make_identity

F32 = mybir.dt.float32
BF16 = mybir.dt.bfloat16
I32 = mybir.dt.int32
AF = mybir.ActivationFunctionType

# Problem constants
B, VD, T_FULL, TD, L, A, G = 32, 1024, 2048, 768, 128, 1024, 8
D = A // G  # 128
N_CORES = 8
B_PC = B // N_CORES  # 4 batches per core
SCALE = 32.0
EXP_C = 24.0  # fixed softmax offset; |att| << 24 for this data distribution


def build_kernel(b_pc: int = B_PC, t: int = T_FULL):
    """Build the per-core Bass program. Returns the compiled Bacc object."""
    nc = bacc.Bacc("TRN2", target_bir_lowering=False, debug=False)

    assert t % 1024 == 0
    nt = t // 128  # T tiles
    nch = t // 512  # 512-wide chunks
    npair = t // 1024  # 1024-wide chunk pairs
    kv = VD // 128  # 8 k-tiles over video dim
    kt = TD // 128  # 6 k-tiles over text dim

    videofea = nc.dram_tensor("videofea", [b_pc, VD, t], F32, kind="ExternalInput").ap()
    textfea = nc.dram_tensor("textfea", [b_pc, L, TD], F32, kind="ExternalInput").ap()
    mask = nc.dram_tensor("mask", [b_pc, t, L], I32, kind="ExternalInput").ap()
    wq = nc.dram_tensor("Wq", [TD, A], F32, kind="ExternalInput").ap()
    bq = nc.dram_tensor("bq", [A], F32, kind="ExternalInput").ap()
    wk = nc.dram_tensor("Wk", [VD, A], F32, kind="ExternalInput").ap()
    bk = nc.dram_tensor("bk", [A], F32, kind="ExternalInput").ap()
    wvv = nc.dram_tensor("Wvv", [TD, A], F32, kind="ExternalInput").ap()
    bvv = nc.dram_tensor("bvv", [A], F32, kind="ExternalInput").ap()
    wvt = nc.dram_tensor("Wvt", [VD, A], F32, kind="ExternalInput").ap()
    bvt = nc.dram_tensor("bvt", [A], F32, kind="ExternalInput").ap()

    # out_v in the reference's [b, g*d, T] layout directly
    out_v = nc.dram_tensor("out_v", [b_pc, A, t], F32, kind="ExternalOutput").ap()
    out_t = nc.dram_tensor("out_t", [b_pc, L, A], F32, kind="ExternalOutput").ap()

    # internal DRAM scratch
    m_scr = nc.dram_tensor("m_scr", [b_pc, t, L], BF16).ap()  # mask transpose bounce
    kt_scr = nc.dram_tensor("kt_scr", [b_pc, G, 128, t], F32).ap()  # f32 kT spill
    rv_scr = nc.dram_tensor("rv_scr", [G, t], F32).ap()  # colsum recip bounce

    def bcast_ap(vec: bass.AP, parts: int = 128) -> bass.AP:
        # [N] dram vector -> [parts, N] partition-broadcast AP (for DMA)
        return bass.AP(tensor=vec.tensor, offset=vec.offset, ap=[[0, parts]] + list(vec.ap))

    from contextlib import ExitStack

    with tile.TileContext(nc) as tc:
        with ExitStack() as ctx:
            ec = ctx.enter_context
            consts = ec(tc.tile_pool(name="consts", bufs=1))
            vid_pool = ec(tc.tile_pool(name="vid", bufs=1))
            txt_pool = ec(tc.tile_pool(name="txt", bufs=1))
            wqs_pool = ec(tc.tile_pool(name="wqs", bufs=2))
            vt_pool = ec(tc.tile_pool(name="vt", bufs=1))
            ktg_pool = ec(tc.tile_pool(name="ktg", bufs=2))
            masktl_pool = ec(tc.tile_pool(name="masktl", bufs=1))
            masklt_pool = ec(tc.tile_pool(name="masklt", bufs=1))
            qv_pool = ec(tc.tile_pool(name="qv", bufs=1))
            p_pool = ec(tc.tile_pool(name="p", bufs=2))
            pt_pool = ec(tc.tile_pool(name="pt", bufs=2))
            wide_pool = ec(tc.tile_pool(name="wide", bufs=1))  # kT stage
            rvb_pool = ec(tc.tile_pool(name="rvb", bufs=2))
            small_pool = ec(tc.tile_pool(name="small", bufs=4))
            ostage_pool = ec(tc.tile_pool(name="ostage", bufs=1))
            ov_pool = ec(tc.tile_pool(name="ovstage", bufs=1))
            ps_bank = ec(tc.tile_pool(name="ps_bank", bufs=2, space="PSUM"))  # 2-bank
            ps_acc = ec(tc.tile_pool(name="ps_acc", bufs=4, space="PSUM"))  # 1-bank

            # ---------------- constants ----------------
            wvv_sb = consts.tile([128, kt, A], BF16)
            nc.gpsimd.dma_start(out=wvv_sb, in_=wvv.rearrange("(k p) a -> p k a", p=128))
            wk_sb = consts.tile([128, kv, A], BF16)
            nc.gpsimd.dma_start(out=wk_sb, in_=wk.rearrange("(k p) a -> p k a", p=128))
            wvt_sb = consts.tile([128, kv, A], BF16)
            nc.gpsimd.dma_start(out=wvt_sb, in_=wvt.rearrange("(k p) a -> p k a", p=128))

            bk_sb = consts.tile([128, G], F32)
            nc.sync.dma_start(out=bk_sb, in_=bk.rearrange("(g p) -> p g", p=128))
            # bq as a single-partition row for the K=1 bias matmul
            bq_row = consts.tile([1, A], F32)
            nc.sync.dma_start(out=bq_row, in_=bq[None, :])
            ones_row = consts.tile([1, L], F32)
            nc.vector.memset(ones_row, 1.0)

            # bvt/SCALE and bvv/SCALE broadcast along partitions
            bvt_sb = consts.tile([128, A], BF16)
            nc.gpsimd.dma_start(out=bvt_sb, in_=bcast_ap(bvt))
            nc.vector.tensor_scalar_mul(bvt_sb, bvt_sb, 1.0 / SCALE)
            bvv_sb = consts.tile([128, A], BF16)
            nc.gpsimd.dma_start(out=bvv_sb, in_=bcast_ap(bvv))
            nc.vector.tensor_scalar_mul(bvv_sb, bvv_sb, 1.0 / SCALE)

            identity = consts.tile([128, 128], F32)
            make_identity(nc, identity)

            neg_c = consts.tile([128, 1], F32)
            nc.vector.memset(neg_c, -EXP_C)

            for bi in range(b_pc):
                # ---------------- per-batch loads ----------------
                video_sb = vid_pool.tile([128, kv, t], BF16, tag="video")
                nc.gpsimd.dma_start(
                    out=video_sb, in_=videofea[bi].rearrange("(k p) t -> p k t", p=128)
                )
                text_sb = txt_pool.tile([128, TD], F32, tag="text")
                nc.sync.dma_start(out=text_sb, in_=textfea[bi])

                # mask: int32 [T, L] --cast--> bf16, bounce through DRAM for the
                # xbar transpose into [L, T]
                m_tl = masktl_pool.tile([128, nt, L], BF16, tag="mtl")
                nc.gpsimd.dma_start(
                    out=m_tl, in_=mask[bi].rearrange("(tt p) l -> p tt l", p=128)
                )
                nc.sync.dma_start(
                    out=m_scr[bi].rearrange("(tt p) l -> p tt l", p=128), in_=m_tl
                )
                m_lt = masklt_pool.tile([L, t], BF16, tag="mlt")
                nc.sync.dma_start_transpose(out=m_lt, in_=m_scr[bi])

                # textfea^T via PE transpose: f32 feeds q, bf16 feeds vv
                textT_sb = txt_pool.tile([128, kt, L], F32, tag="textT")
                textT_bf = txt_pool.tile([128, kt, L], BF16, tag="textT_bf")
                for k in range(kt):
                    ps_tr = ps_acc.tile([128, 128], F32, tag="acc")
                    nc.tensor.transpose(ps_tr, text_sb[:, bass.ts(k, 128)], identity)
                    nc.vector.tensor_copy(out=textT_sb[:, k], in_=ps_tr)
                    nc.vector.tensor_copy(out=textT_bf[:, k], in_=ps_tr)

                # ---------------- projections ----------------
                # qT [A(g), L] f32; Wq streamed from DRAM per k-tile (k-outer);
                # one [128, 8*L] psum tile held across the k loop; bq added via
                # a K=1 matmul against a ones row. Only the very first matmul
                # sets start (it clears the whole bank's has_written bits);
                # later first-touches overwrite per element.
                ps_q = ps_bank.tile([128, G, L], F32, tag="bank", name="ps_q")
                nc.vector.memset(ps_q, 0.0)
                for k in range(kt):
                    wq_k = wqs_pool.tile([128, A], F32, tag="wq")
                    nc.sync.dma_start(out=wq_k, in_=wq[bass.ts(k, 128), :])
                    for g in range(G):
                        nc.tensor.matmul(
                            ps_q[:, g],
                            lhsT=wq_k[:, bass.ts(g, 128)],
                            rhs=textT_sb[:, k],
                            start=False,
                            stop=False,
                            skip_group_check=True,
                        )
                qT_sb = qv_pool.tile([128, G, L], F32, tag="qT")
                for g in range(G):
                    nc.tensor.matmul(
                        ps_q[:, g],
                        lhsT=bq_row[:, bass.ts(g, 128)],
                        rhs=ones_row,
                        start=False,
                        stop=(g == G - 1),
                        skip_group_check=True,
                    )
                nc.vector.tensor_copy(out=qT_sb, in_=ps_q)

                # vv [L, A] bf16, pre-scaled by 1/32, bvv/32 added
                vv_sb = qv_pool.tile([128, A], BF16, tag="vv")
                for half in range(2):
                    ps = ps_acc.tile([128, 512], F32, tag="acc")
                    for k in range(kt):
                        nc.tensor.matmul(
                            ps,
                            lhsT=textT_bf[:, k],
                            rhs=wvv_sb[:, k, bass.ts(half, 512)],
                            start=(k == 0),
                            stop=(k == kt - 1),
                        )
                    nc.vector.scalar_tensor_tensor(
                        out=vv_sb[:, bass.ts(half, 512)],
                        in0=ps,
                        scalar=1.0 / SCALE,
                        op0=AluOpType.mult,
                        in1=bvv_sb[:, bass.ts(half, 512)],
                        op1=AluOpType.add,
                    )

                # vt [T, A] bf16 resident, pre-scaled by 1/32
                vt_sb = vt_pool.tile([128, nt, A], BF16, tag="vt")
                for mt in range(nt):
                    ps = ps_bank.tile([128, A], F32, tag="bank")
                    for k in range(kv):
                        for half in range(2):
                            nc.tensor.matmul(
                                ps[:, bass.ts(half, 512)],
                                lhsT=video_sb[:, k, bass.ts(mt, 128)],
                                rhs=wvt_sb[:, k, bass.ts(half, 512)],
                                start=(k == 0),
                                stop=(k == kv - 1),
                            )
                    nc.vector.tensor_scalar(
                        out=vt_sb[:, mt], in0=ps, scalar1=1.0 / SCALE,
                        scalar2=None, op0=AluOpType.mult,
                    )

                # kT [A(g), T] f32 with bias; spilled to DRAM, re-read per group
                for g in range(G):
                    for pair in range(npair):
                        ps = ps_bank.tile([128, 1024], F32, tag="bank")
                        for k in range(kv):
                            for ch in range(2):
                                nc.tensor.matmul(
                                    ps[:, bass.ts(ch, 512)],
                                    lhsT=wk_sb[:, k, bass.ts(g, 128)],
                                    rhs=video_sb[:, k, bass.ds(pair * 1024 + ch * 512, 512)],
                                    start=(k == 0),
                                    stop=(k == kv - 1),
                                )
                        k_stage = wide_pool.tile([128, 1024], F32, tag="wide")
                        nc.vector.tensor_scalar(
                            out=k_stage, in0=ps, scalar1=bk_sb[:, g : g + 1],
                            scalar2=None, op0=AluOpType.add,
                        )
                        nc.sync.dma_start(
                            out=kt_scr[bi, g, :, bass.ts(pair, 1024)], in_=k_stage
                        )

                # -------- per-group attention (skew-1 software pipeline) -----
                state = {}

                def phase_a(g):
                    st = {}
                    kT_g = ktg_pool.tile([128, t], F32, tag="kT_g", name="kT_g")
                    nc.sync.dma_start(out=kT_g, in_=kt_scr[bi, g])

                    # S [L, T] f32; exp chunks; one wide mask-mult
                    p_sb = p_pool.tile([L, t], BF16, tag="P", name="p_sb")
                    for ch in range(nch):
                        ps = ps_acc.tile([128, 512], F32, tag="acc", name="ps_s")
                        nc.tensor.matmul(
                            ps, lhsT=qT_sb[:, g], rhs=kT_g[:, bass.ts(ch, 512)],
                            start=True, stop=True,
                        )
                        nc.scalar.activation(
                            out=p_sb[:, bass.ts(ch, 512)], in_=ps, func=AF.Exp,
                            bias=neg_c,
                        )
                    nc.vector.tensor_tensor(
                        out=p_sb, in0=p_sb, in1=m_lt, op=AluOpType.mult
                    )
                    rs = small_pool.tile([128, 1], F32, tag="rs", name="rs")
                    nc.vector.tensor_reduce(
                        out=rs, in_=p_sb, axis=mybir.AxisListType.X, op=AluOpType.add
                    )
                    recip_t = small_pool.tile([128, 1], F32, tag="recip_t", name="rt")
                    nc.vector.reciprocal(out=recip_t, in_=rs)

                    # S^T [T, L] tiles; exp + mask per 4-tile chunk; colsums
                    pt_sb = pt_pool.tile([128, nt, L], BF16, tag="PT", name="pt_sb")
                    cs = small_pool.tile([128, nt], F32, tag="cs", name="cs")
                    for grp in range(nch):
                        ps = ps_acc.tile([128, 4, 128], F32, tag="acc", name="ps_st")
                        for j in range(4):
                            tt = grp * 4 + j
                            nc.tensor.matmul(
                                ps[:, j],
                                lhsT=kT_g[:, bass.ts(tt, 128)],
                                rhs=qT_sb[:, g],
                                start=(j == 0),
                                stop=(j == 3),
                                skip_group_check=True,
                            )
                        sl = slice(grp * 4, grp * 4 + 4)
                        nc.scalar.activation(
                            out=pt_sb[:, sl], in_=ps, func=AF.Exp, bias=neg_c
                        )
                        nc.vector.tensor_tensor(
                            out=pt_sb[:, sl], in0=pt_sb[:, sl], in1=m_tl[:, sl],
                            op=AluOpType.mult,
                        )
                        nc.vector.tensor_reduce(
                            out=cs[:, sl], in_=pt_sb[:, sl],
                            axis=mybir.AxisListType.X, op=AluOpType.add,
                        )
                    rv = small_pool.tile([128, nt], F32, tag="rv", name="rv")
                    nc.vector.reciprocal(out=rv, in_=cs)
                    nc.sync.dma_start(
                        out=rv_scr[g].rearrange("(tt p) -> p tt", p=128), in_=rv
                    )
                    rvb = rvb_pool.tile([128, t], F32, tag="rvb", name="rvb")
                    nc.sync.dma_start(out=rvb, in_=bcast_ap(rv_scr[g]))

                    st["p_sb"], st["pt_sb"] = p_sb, pt_sb
                    st["recip_t"], st["rvb"] = recip_t, rvb
                    state[g] = st

                def phase_b(g):
                    st = state.pop(g)
                    p_sb, pt_sb = st["p_sb"], st["pt_sb"]

                    # out_t[L, d_g] = (PT.T @ vt_g) * recip_t + bvt_g/32
                    ps_ot = ps_acc.tile([128, 128], F32, tag="acc", name="ps_ot")
                    for tt in range(nt):
                        nc.tensor.matmul(
                            ps_ot,
                            lhsT=pt_sb[:, tt],
                            rhs=vt_sb[:, tt, bass.ts(g, 128)],
                            start=(tt == 0),
                            stop=(tt == nt - 1),
                        )
                    ot = ostage_pool.tile([128, 128], F32, tag="ot", name="ot")
                    nc.vector.scalar_tensor_tensor(
                        out=ot,
                        in0=ps_ot,
                        scalar=st["recip_t"],
                        op0=AluOpType.mult,
                        in1=bvt_sb[:, bass.ts(g, 128)],
                        op1=AluOpType.add,
                    )
                    nc.sync.dma_start(out=out_t[bi, :, bass.ts(g, 128)], in_=ot)

                    # out_v^T[d_g, T] = vv_g.T @ P ; column normalizer applied
                    # at the PSUM->SBUF copy via the broadcast reciprocal
                    ovT = ov_pool.tile([128, t], F32, tag="ovT", name="ovT")
                    for pair in range(npair):
                        ps = ps_bank.tile([128, 1024], F32, tag="bank", name="ps_ov")
                        for ch in range(2):
                            nc.tensor.matmul(
                                ps[:, bass.ts(ch, 512)],
                                lhsT=vv_sb[:, bass.ts(g, 128)],
                                rhs=p_sb[:, bass.ds(pair * 1024 + ch * 512, 512)],
                                start=True,
                                stop=True,
                            )
                        nc.vector.tensor_tensor(
                            out=ovT[:, bass.ts(pair, 1024)],
                            in0=ps,
                            in1=st["rvb"][:, bass.ts(pair, 1024)],
                            op=AluOpType.mult,
                        )
                    nc.sync.dma_start(out=out_v[bi, bass.ts(g, 128), :], in_=ovT)

                for g in range(G):
                    phase_a(g)
                    if g >= 1:
                        phase_b(g - 1)
                phase_b(G - 1)

    nc.compile()
    return nc


_NC_CACHE: dict = {}


def _get_nc():
    if "nc" not in _NC_CACHE:
        _NC_CACHE["nc"] = build_kernel()
    return _NC_CACHE["nc"]


def kernel(**inputs) -> tuple:
    nc = _get_nc()
    in_maps = []
    for c in range(N_CORES):
        sl = slice(c * B_PC, (c + 1) * B_PC)
        in_maps.append(
            {
                "videofea": np.ascontiguousarray(inputs["videofea"][sl]),
                "textfea": np.ascontiguousarray(inputs["textfea"][sl]),
                "mask": np.ascontiguousarray(inputs["mask"][sl]),
                "Wq": np.asarray(inputs["Wq"]),
                "bq": np.asarray(inputs["bq"]),
                "Wk": np.asarray(inputs["Wk"]),
                "bk": np.asarray(inputs["bk"]),
                "Wvv": np.asarray(inputs["Wvv"]),
                "bvv": np.asarray(inputs["bvv"]),
                "Wvt": np.asarray(inputs["Wvt"]),
                "bvt": np.asarray(inputs["bvt"]),
            }
        )
    res = run_bass_kernel_spmd(nc, in_maps, core_ids=list(range(N_CORES)))
    out_v = np.concatenate([r["out_v"] for r in res.results], axis=0)  # [B, A, T]
    out_t = np.concatenate([r["out_t"] for r in res.results], axis=0)  # [B, L, A]
    return out_v, out_t


# revision 31
# speedup vs baseline: 1.1635x; 1.0836x over previous
"""Trainium2 Bass kernel for nn_AttentionCross (dual-direction masked cross attention).

Computation per batch b (reference semantics):
    v   = videofea.T                      [T, vd]
    q   = split_heads(textfea @ Wq + bq)  [g, L, d]
    k   = split_heads(v @ Wk + bk)        [g, T, d]
    vv  = split_heads(textfea @ Wvv+bvv)  [g, L, d]
    vt  = split_heads(v @ Wvt + bvt)      [g, T, d]
    att = q @ k.T (masked; -1e9 where mask==0)        [g, L, T]
    att_t = softmax_T(att)/32 ; att_v = softmax_L(att.T)/32
    out_v = att_v @ vv   -> [b, g*d, T]
    out_t = att_t @ vt   -> [b, L, g*d]

Strategy: data-parallel over batch across 8 NeuronCores (4 batches/core).

Numerics: projections run in bf16 except the q path (f32) and the S matmuls
(f32 with f32-stored q,k) — rounding of the exp() argument is the dominant
error term, so k is produced in f32 (spilled to DRAM; too big for SBUF) and
q in f32. Softmax uses a fixed offset C (exact after normalization; value
range is bounded). The 1/sqrt(dim) output scale is pre-folded into vt/vv.

The attention matrix is materialized in both layouts by two matmul passes
(S in [L,T] for out_v's operand; S^T in [T,L] tiles for out_t's T-contraction)
— each pass exp'd and masked in its own layout. All softmax reductions are
free-dim reductions (on GpSimd, which is otherwise idle); normalizations are
per-output-partition scales except out_v's per-column one, which is applied
via a partition-broadcast reciprocal tile at the PSUM->SBUF copy (off the PE
critical path). out_v is computed as out_v^T = vv^T @ P (one weight load,
N=512 moving operand) and lands in the reference's [b, g*d, T] layout with
fully contiguous DMA. The group loop is software-pipelined with skew 1 so
the PE never waits on the exp/mask lag of the current group.

bvt is folded in post-hoc: sum_t att_t[l,t] == 1/32 exactly, so
out_t += bvt/32 after the matmul; bvv is added into vv directly.
"""

import sys

if "/opt/trn_rl_repo" not in sys.path:
    sys.path.insert(0, "/opt/trn_rl_repo")

import numpy as np

import concourse.bass as bass
import concourse.mybir as mybir
import concourse.tile as tile
from concourse import bacc
from concourse.alu_op_type import AluOpType
from concourse.bass_utils import run_bass_kernel_spmd
from concourse.masks import make_identity

F32 = mybir.dt.float32
BF16 = mybir.dt.bfloat16
I32 = mybir.dt.int32
AF = mybir.ActivationFunctionType

# Problem constants
B, VD, T_FULL, TD, L, A, G = 32, 1024, 2048, 768, 128, 1024, 8
D = A // G  # 128
N_CORES = 8
B_PC = B // N_CORES  # 4 batches per core
SCALE = 32.0
EXP_C = 24.0  # fixed softmax offset; |att| << 24 for this data distribution


def build_kernel(b_pc: int = B_PC, t: int = T_FULL):
    """Build the per-core Bass program. Returns the compiled Bacc object."""
    nc = bacc.Bacc("TRN2", target_bir_lowering=False, debug=False)

    assert t % 1024 == 0
    nt = t // 128  # T tiles
    nch = t // 512  # 512-wide chunks
    npair = t // 1024  # 1024-wide chunk pairs
    kv = VD // 128  # 8 k-tiles over video dim
    kt = TD // 128  # 6 k-tiles over text dim

    videofea = nc.dram_tensor("videofea", [b_pc, VD, t], F32, kind="ExternalInput").ap()
    textfea = nc.dram_tensor("textfea", [b_pc, L, TD], F32, kind="ExternalInput").ap()
    mask = nc.dram_tensor("mask", [b_pc, t, L], I32, kind="ExternalInput").ap()
    wq = nc.dram_tensor("Wq", [TD, A], F32, kind="ExternalInput").ap()
    bq = nc.dram_tensor("bq", [A], F32, kind="ExternalInput").ap()
    wk = nc.dram_tensor("Wk", [VD, A], F32, kind="ExternalInput").ap()
    bk = nc.dram_tensor("bk", [A], F32, kind="ExternalInput").ap()
    wvv = nc.dram_tensor("Wvv", [TD, A], F32, kind="ExternalInput").ap()
    bvv = nc.dram_tensor("bvv", [A], F32, kind="ExternalInput").ap()
    wvt = nc.dram_tensor("Wvt", [VD, A], F32, kind="ExternalInput").ap()
    bvt = nc.dram_tensor("bvt", [A], F32, kind="ExternalInput").ap()

    # out_v in the reference's [b, g*d, T] layout directly
    out_v = nc.dram_tensor("out_v", [b_pc, A, t], F32, kind="ExternalOutput").ap()
    out_t = nc.dram_tensor("out_t", [b_pc, L, A], F32, kind="ExternalOutput").ap()

    # internal DRAM scratch
    m_scr = nc.dram_tensor("m_scr", [b_pc, t, L], BF16).ap()  # mask transpose bounce
    rv_scr = nc.dram_tensor("rv_scr", [G, t], F32).ap()  # colsum recip bounce

    def bcast_ap(vec: bass.AP, parts: int = 128) -> bass.AP:
        # [N] dram vector -> [parts, N] partition-broadcast AP (for DMA)
        return bass.AP(tensor=vec.tensor, offset=vec.offset, ap=[[0, parts]] + list(vec.ap))

    from contextlib import ExitStack

    with tile.TileContext(nc) as tc:
        with ExitStack() as ctx:
            ec = ctx.enter_context
            consts = ec(tc.tile_pool(name="consts", bufs=1))
            vid_pool = ec(tc.tile_pool(name="vid", bufs=1))
            txt_pool = ec(tc.tile_pool(name="txt", bufs=1))
            wqs_pool = ec(tc.tile_pool(name="wqs", bufs=2))
            vt_pool = ec(tc.tile_pool(name="vt", bufs=1))
            ktg_pool = ec(tc.tile_pool(name="ktg", bufs=2))
            masktl_pool = ec(tc.tile_pool(name="masktl", bufs=1))
            masklt_pool = ec(tc.tile_pool(name="masklt", bufs=1))
            qv_pool = ec(tc.tile_pool(name="qv", bufs=1))
            p_pool = ec(tc.tile_pool(name="p", bufs=2))
            pt_pool = ec(tc.tile_pool(name="pt", bufs=2))
            rvb_pool = ec(tc.tile_pool(name="rvb", bufs=2))
            small_pool = ec(tc.tile_pool(name="small", bufs=4))
            ostage_pool = ec(tc.tile_pool(name="ostage", bufs=1))
            ov_pool = ec(tc.tile_pool(name="ovstage", bufs=1))
            ps_bank = ec(tc.tile_pool(name="ps_bank", bufs=2, space="PSUM"))  # 2-bank
            ps_acc = ec(tc.tile_pool(name="ps_acc", bufs=4, space="PSUM"))  # 1-bank

            # ---------------- constants ----------------
            wvv_sb = consts.tile([128, kt, A], BF16)
            nc.gpsimd.dma_start(out=wvv_sb, in_=wvv.rearrange("(k p) a -> p k a", p=128))
            wk_sb = consts.tile([128, kv, A], BF16)
            nc.gpsimd.dma_start(out=wk_sb, in_=wk.rearrange("(k p) a -> p k a", p=128))
            wvt_sb = consts.tile([128, kv, A], BF16)
            nc.gpsimd.dma_start(out=wvt_sb, in_=wvt.rearrange("(k p) a -> p k a", p=128))

            bk_sb = consts.tile([128, G], F32)
            nc.sync.dma_start(out=bk_sb, in_=bk.rearrange("(g p) -> p g", p=128))
            # bq as a single-partition row for the K=1 bias matmul
            bq_row = consts.tile([1, A], F32)
            nc.sync.dma_start(out=bq_row, in_=bq[None, :])
            ones_row = consts.tile([1, L], F32)
            nc.vector.memset(ones_row, 1.0)

            # bvt/SCALE and bvv/SCALE broadcast along partitions
            bvt_sb = consts.tile([128, A], BF16)
            nc.gpsimd.dma_start(out=bvt_sb, in_=bcast_ap(bvt))
            nc.vector.tensor_scalar_mul(bvt_sb, bvt_sb, 1.0 / SCALE)
            bvv_sb = consts.tile([128, A], BF16)
            nc.gpsimd.dma_start(out=bvv_sb, in_=bcast_ap(bvv))
            nc.vector.tensor_scalar_mul(bvv_sb, bvv_sb, 1.0 / SCALE)

            identity = consts.tile([128, 128], F32)
            make_identity(nc, identity)

            neg_c = consts.tile([128, 1], F32)
            nc.vector.memset(neg_c, -EXP_C)

            for bi in range(b_pc):
                # ---------------- per-batch loads ----------------
                video_sb = vid_pool.tile([128, kv, t], BF16, tag="video")
                nc.gpsimd.dma_start(
                    out=video_sb, in_=videofea[bi].rearrange("(k p) t -> p k t", p=128)
                )
                text_sb = txt_pool.tile([128, TD], F32, tag="text")
                nc.sync.dma_start(out=text_sb, in_=textfea[bi])

                # mask: int32 [T, L] --cast--> bf16, bounce through DRAM for the
                # xbar transpose into [L, T]
                m_tl = masktl_pool.tile([128, nt, L], BF16, tag="mtl")
                nc.gpsimd.dma_start(
                    out=m_tl, in_=mask[bi].rearrange("(tt p) l -> p tt l", p=128)
                )
                nc.sync.dma_start(
                    out=m_scr[bi].rearrange("(tt p) l -> p tt l", p=128), in_=m_tl
                )
                m_lt = masklt_pool.tile([L, t], BF16, tag="mlt")
                nc.sync.dma_start_transpose(out=m_lt, in_=m_scr[bi])

                # textfea^T via PE transpose: f32 feeds q, bf16 feeds vv
                textT_sb = txt_pool.tile([128, kt, L], F32, tag="textT")
                textT_bf = txt_pool.tile([128, kt, L], BF16, tag="textT_bf")
                for k in range(kt):
                    ps_tr = ps_acc.tile([128, 128], F32, tag="acc")
                    nc.tensor.transpose(ps_tr, text_sb[:, bass.ts(k, 128)], identity)
                    nc.vector.tensor_copy(out=textT_sb[:, k], in_=ps_tr)
                    nc.vector.tensor_copy(out=textT_bf[:, k], in_=ps_tr)

                # ---------------- projections ----------------
                # qT [A(g), L] f32; Wq streamed from DRAM per k-tile (k-outer);
                # one [128, 8*L] psum tile held across the k loop; bq added via
                # a K=1 matmul against a ones row. Only the very first matmul
                # sets start (it clears the whole bank's has_written bits);
                # later first-touches overwrite per element.
                ps_q = ps_bank.tile([128, G, L], F32, tag="bank", name="ps_q")
                nc.vector.memset(ps_q, 0.0)
                for k in range(kt):
                    wq_k = wqs_pool.tile([128, A], F32, tag="wq")
                    nc.sync.dma_start(out=wq_k, in_=wq[bass.ts(k, 128), :])
                    for g in range(G):
                        nc.tensor.matmul(
                            ps_q[:, g],
                            lhsT=wq_k[:, bass.ts(g, 128)],
                            rhs=textT_sb[:, k],
                            start=False,
                            stop=False,
                            skip_group_check=True,
                        )
                qT_sb = qv_pool.tile([128, G, L], F32, tag="qT")
                for g in range(G):
                    nc.tensor.matmul(
                        ps_q[:, g],
                        lhsT=bq_row[:, bass.ts(g, 128)],
                        rhs=ones_row,
                        start=False,
                        stop=(g == G - 1),
                        skip_group_check=True,
                    )
                nc.vector.tensor_copy(out=qT_sb, in_=ps_q)

                # vv [L, A] bf16, pre-scaled by 1/32, bvv/32 added
                vv_sb = qv_pool.tile([128, A], BF16, tag="vv")
                for half in range(2):
                    ps = ps_acc.tile([128, 512], F32, tag="acc")
                    for k in range(kt):
                        nc.tensor.matmul(
                            ps,
                            lhsT=textT_bf[:, k],
                            rhs=wvv_sb[:, k, bass.ts(half, 512)],
                            start=(k == 0),
                            stop=(k == kt - 1),
                        )
                    nc.vector.scalar_tensor_tensor(
                        out=vv_sb[:, bass.ts(half, 512)],
                        in0=ps,
                        scalar=1.0 / SCALE,
                        op0=AluOpType.mult,
                        in1=bvv_sb[:, bass.ts(half, 512)],
                        op1=AluOpType.add,
                    )

                # vt [T, A] bf16 resident, pre-scaled by 1/32
                vt_sb = vt_pool.tile([128, nt, A], BF16, tag="vt")
                for mt in range(nt):
                    ps = ps_bank.tile([128, A], F32, tag="bank")
                    for k in range(kv):
                        for half in range(2):
                            nc.tensor.matmul(
                                ps[:, bass.ts(half, 512)],
                                lhsT=video_sb[:, k, bass.ts(mt, 128)],
                                rhs=wvt_sb[:, k, bass.ts(half, 512)],
                                start=(k == 0),
                                stop=(k == kv - 1),
                            )
                    nc.vector.tensor_scalar(
                        out=vt_sb[:, mt], in0=ps, scalar1=1.0 / SCALE,
                        scalar2=None, op0=AluOpType.mult,
                    )

                # -------- per-group attention (skew-1 software pipeline) -----
                state = {}

                def phase_a(g):
                    st = {}
                    # kT_g [d, T] f32 recomputed from resident video (no DRAM
                    # spill: saves 64MB/core of queue traffic + read latency)
                    kT_g = ktg_pool.tile([128, t], F32, tag="kT_g", name="kT_g")
                    for ch in range(nch):
                        ps = ps_acc.tile([128, 512], F32, tag="acc", name="ps_kt")
                        for k in range(kv):
                            nc.tensor.matmul(
                                ps,
                                lhsT=wk_sb[:, k, bass.ts(g, 128)],
                                rhs=video_sb[:, k, bass.ts(ch, 512)],
                                start=(k == 0),
                                stop=(k == kv - 1),
                            )
                        nc.vector.tensor_scalar(
                            out=kT_g[:, bass.ts(ch, 512)], in0=ps,
                            scalar1=bk_sb[:, g : g + 1], scalar2=None,
                            op0=AluOpType.add,
                        )

                    # S [L, T] f32; exp chunks; one wide mask-mult
                    p_sb = p_pool.tile([L, t], BF16, tag="P", name="p_sb")
                    for ch in range(nch):
                        ps = ps_acc.tile([128, 512], F32, tag="acc", name="ps_s")
                        nc.tensor.matmul(
                            ps, lhsT=qT_sb[:, g], rhs=kT_g[:, bass.ts(ch, 512)],
                            start=True, stop=True,
                        )
                        nc.scalar.activation(
                            out=p_sb[:, bass.ts(ch, 512)], in_=ps, func=AF.Exp,
                            bias=neg_c,
                        )
                    nc.vector.tensor_tensor(
                        out=p_sb, in0=p_sb, in1=m_lt, op=AluOpType.mult
                    )
                    rs = small_pool.tile([128, 1], F32, tag="rs", name="rs")
                    nc.vector.tensor_reduce(
                        out=rs, in_=p_sb, axis=mybir.AxisListType.X, op=AluOpType.add
                    )
                    recip_t = small_pool.tile([128, 1], F32, tag="recip_t", name="rt")
                    nc.vector.reciprocal(out=recip_t, in_=rs)

                    # S^T [T, L] tiles; exp + mask per 4-tile chunk; colsums
                    pt_sb = pt_pool.tile([128, nt, L], BF16, tag="PT", name="pt_sb")
                    cs = small_pool.tile([128, nt], F32, tag="cs", name="cs")
                    for grp in range(nch):
                        ps = ps_acc.tile([128, 4, 128], F32, tag="acc", name="ps_st")
                        for j in range(4):
                            tt = grp * 4 + j
                            nc.tensor.matmul(
                                ps[:, j],
                                lhsT=kT_g[:, bass.ts(tt, 128)],
                                rhs=qT_sb[:, g],
                                start=(j == 0),
                                stop=(j == 3),
                                skip_group_check=True,
                            )
                        sl = slice(grp * 4, grp * 4 + 4)
                        nc.scalar.activation(
                            out=pt_sb[:, sl], in_=ps, func=AF.Exp, bias=neg_c
                        )
                        nc.vector.tensor_tensor(
                            out=pt_sb[:, sl], in0=pt_sb[:, sl], in1=m_tl[:, sl],
                            op=AluOpType.mult,
                        )
                        nc.vector.tensor_reduce(
                            out=cs[:, sl], in_=pt_sb[:, sl],
                            axis=mybir.AxisListType.X, op=AluOpType.add,
                        )
                    rv = small_pool.tile([128, nt], F32, tag="rv", name="rv")
                    nc.vector.reciprocal(out=rv, in_=cs)
                    nc.sync.dma_start(
                        out=rv_scr[g].rearrange("(tt p) -> p tt", p=128), in_=rv
                    )
                    rvb = rvb_pool.tile([128, t], F32, tag="rvb", name="rvb")
                    nc.sync.dma_start(out=rvb, in_=bcast_ap(rv_scr[g]))

                    st["p_sb"], st["pt_sb"] = p_sb, pt_sb
                    st["recip_t"], st["rvb"] = recip_t, rvb
                    state[g] = st

                def phase_b(g):
                    st = state.pop(g)
                    p_sb, pt_sb = st["p_sb"], st["pt_sb"]

                    # out_t[L, d_g] = (PT.T @ vt_g) * recip_t + bvt_g/32
                    ps_ot = ps_acc.tile([128, 128], F32, tag="acc", name="ps_ot")
                    for tt in range(nt):
                        nc.tensor.matmul(
                            ps_ot,
                            lhsT=pt_sb[:, tt],
                            rhs=vt_sb[:, tt, bass.ts(g, 128)],
                            start=(tt == 0),
                            stop=(tt == nt - 1),
                        )
                    ot = ostage_pool.tile([128, 128], F32, tag="ot", name="ot")
                    nc.vector.scalar_tensor_tensor(
                        out=ot,
                        in0=ps_ot,
                        scalar=st["recip_t"],
                        op0=AluOpType.mult,
                        in1=bvt_sb[:, bass.ts(g, 128)],
                        op1=AluOpType.add,
                    )
                    nc.sync.dma_start(out=out_t[bi, :, bass.ts(g, 128)], in_=ot)

                    # out_v^T[d_g, T] = vv_g.T @ P ; column normalizer applied
                    # at the PSUM->SBUF copy via the broadcast reciprocal
                    ovT = ov_pool.tile([128, t], F32, tag="ovT", name="ovT")
                    for pair in range(npair):
                        ps = ps_bank.tile([128, 1024], F32, tag="bank", name="ps_ov")
                        for ch in range(2):
                            nc.tensor.matmul(
                                ps[:, bass.ts(ch, 512)],
                                lhsT=vv_sb[:, bass.ts(g, 128)],
                                rhs=p_sb[:, bass.ds(pair * 1024 + ch * 512, 512)],
                                start=True,
                                stop=True,
                            )
                        nc.vector.tensor_tensor(
                            out=ovT[:, bass.ts(pair, 1024)],
                            in0=ps,
                            in1=st["rvb"][:, bass.ts(pair, 1024)],
                            op=AluOpType.mult,
                        )
                    nc.sync.dma_start(out=out_v[bi, bass.ts(g, 128), :], in_=ovT)

                for g in range(G):
                    phase_a(g)
                    if g >= 1:
                        phase_b(g - 1)
                phase_b(G - 1)

    nc.compile()
    return nc


_NC_CACHE: dict = {}


def _get_nc():
    if "nc" not in _NC_CACHE:
        _NC_CACHE["nc"] = build_kernel()
    return _NC_CACHE["nc"]


def kernel(**inputs) -> tuple:
    nc = _get_nc()
    in_maps = []
    for c in range(N_CORES):
        sl = slice(c * B_PC, (c + 1) * B_PC)
        in_maps.append(
            {
                "videofea": np.ascontiguousarray(inputs["videofea"][sl]),
                "textfea": np.ascontiguousarray(inputs["textfea"][sl]),
                "mask": np.ascontiguousarray(inputs["mask"][sl]),
                "Wq": np.asarray(inputs["Wq"]),
                "bq": np.asarray(inputs["bq"]),
                "Wk": np.asarray(inputs["Wk"]),
                "bk": np.asarray(inputs["bk"]),
                "Wvv": np.asarray(inputs["Wvv"]),
                "bvv": np.asarray(inputs["bvv"]),
                "Wvt": np.asarray(inputs["Wvt"]),
                "bvt": np.asarray(inputs["bvt"]),
            }
        )
    res = run_bass_kernel_spmd(nc, in_maps, core_ids=list(range(N_CORES)))
    out_v = np.concatenate([r["out_v"] for r in res.results], axis=0)  # [B, A, T]
    out_t = np.concatenate([r["out_t"] for r in res.results], axis=0)  # [B, L, A]
    return out_v, out_t


# revision 32
# speedup vs baseline: 1.3635x; 1.1719x over previous
"""Trainium2 Bass kernel for nn_AttentionCross (dual-direction masked cross attention).

Computation per batch b (reference semantics):
    v   = videofea.T                      [T, vd]
    q   = split_heads(textfea @ Wq + bq)  [g, L, d]
    k   = split_heads(v @ Wk + bk)        [g, T, d]
    vv  = split_heads(textfea @ Wvv+bvv)  [g, L, d]
    vt  = split_heads(v @ Wvt + bvt)      [g, T, d]
    att = q @ k.T (masked; -1e9 where mask==0)        [g, L, T]
    att_t = softmax_T(att)/32 ; att_v = softmax_L(att.T)/32
    out_v = att_v @ vv   -> [b, g*d, T]
    out_t = att_t @ vt   -> [b, L, g*d]

Strategy: data-parallel over batch across 8 NeuronCores (4 batches/core).

Numerics: projections run in bf16 except the q path (f32) and the S matmuls
(f32 with f32-stored q,k) — rounding of the exp() argument is the dominant
error term, so k is produced in f32 (spilled to DRAM; too big for SBUF) and
q in f32. Softmax uses a fixed offset C (exact after normalization; value
range is bounded). The 1/sqrt(dim) output scale is pre-folded into vt/vv.

The attention matrix is materialized in both layouts by two matmul passes
(S in [L,T] for out_v's operand; S^T in [T,L] tiles for out_t's T-contraction)
— each pass exp'd and masked in its own layout. All softmax reductions are
free-dim reductions (on GpSimd, which is otherwise idle); normalizations are
per-output-partition scales except out_v's per-column one, which is applied
via a partition-broadcast reciprocal tile at the PSUM->SBUF copy (off the PE
critical path). out_v is computed as out_v^T = vv^T @ P (one weight load,
N=512 moving operand) and lands in the reference's [b, g*d, T] layout with
fully contiguous DMA. The group loop is software-pipelined with skew 1 so
the PE never waits on the exp/mask lag of the current group.

bvt is folded in post-hoc: sum_t att_t[l,t] == 1/32 exactly, so
out_t += bvt/32 after the matmul; bvv is added into vv directly.
"""

import sys

if "/opt/trn_rl_repo" not in sys.path:
    sys.path.insert(0, "/opt/trn_rl_repo")

import numpy as np

import concourse.bass as bass
import concourse.mybir as mybir
import concourse.tile as tile
from concourse import bacc
from concourse.alu_op_type import AluOpType
from concourse.bass_utils import run_bass_kernel_spmd
from concourse.masks import make_identity

F32 = mybir.dt.float32
BF16 = mybir.dt.bfloat16
I32 = mybir.dt.int32
AF = mybir.ActivationFunctionType

# Problem constants
B, VD, T_FULL, TD, L, A, G = 32, 1024, 2048, 768, 128, 1024, 8
D = A // G  # 128
N_CORES = 8
B_PC = B // N_CORES  # 4 batches per core
SCALE = 32.0
EXP_C = 24.0  # fixed softmax offset; |att| << 24 for this data distribution


def build_kernel(b_pc: int = B_PC, t: int = T_FULL):
    """Build the per-core Bass program. Returns the compiled Bacc object."""
    nc = bacc.Bacc("TRN2", target_bir_lowering=False, debug=False)

    assert t % 1024 == 0
    nt = t // 128  # T tiles
    nch = t // 512  # 512-wide chunks
    npair = t // 1024  # 1024-wide chunk pairs
    kv = VD // 128  # 8 k-tiles over video dim
    kt = TD // 128  # 6 k-tiles over text dim

    videofea = nc.dram_tensor("videofea", [b_pc, VD, t], F32, kind="ExternalInput").ap()
    textfea = nc.dram_tensor("textfea", [b_pc, L, TD], F32, kind="ExternalInput").ap()
    mask = nc.dram_tensor("mask", [b_pc, t, L], I32, kind="ExternalInput").ap()
    wq = nc.dram_tensor("Wq", [TD, A], F32, kind="ExternalInput").ap()
    bq = nc.dram_tensor("bq", [A], F32, kind="ExternalInput").ap()
    wk = nc.dram_tensor("Wk", [VD, A], F32, kind="ExternalInput").ap()
    bk = nc.dram_tensor("bk", [A], F32, kind="ExternalInput").ap()
    wvv = nc.dram_tensor("Wvv", [TD, A], F32, kind="ExternalInput").ap()
    bvv = nc.dram_tensor("bvv", [A], F32, kind="ExternalInput").ap()
    wvt = nc.dram_tensor("Wvt", [VD, A], F32, kind="ExternalInput").ap()
    bvt = nc.dram_tensor("bvt", [A], F32, kind="ExternalInput").ap()

    # out_v in the reference's [b, g*d, T] layout directly
    out_v = nc.dram_tensor("out_v", [b_pc, A, t], F32, kind="ExternalOutput").ap()
    out_t = nc.dram_tensor("out_t", [b_pc, L, A], F32, kind="ExternalOutput").ap()

    # internal DRAM scratch
    m_scr = nc.dram_tensor("m_scr", [b_pc, t, L], BF16).ap()  # mask transpose bounce
    rv_scr = nc.dram_tensor("rv_scr", [G, t], F32).ap()  # colsum recip bounce

    def bcast_ap(vec: bass.AP, parts: int = 128) -> bass.AP:
        # [N] dram vector -> [parts, N] partition-broadcast AP (for DMA)
        return bass.AP(tensor=vec.tensor, offset=vec.offset, ap=[[0, parts]] + list(vec.ap))

    from contextlib import ExitStack

    with tile.TileContext(nc) as tc:
        with ExitStack() as ctx:
            ec = ctx.enter_context
            consts = ec(tc.tile_pool(name="consts", bufs=1))
            vid_pool = ec(tc.tile_pool(name="vid", bufs=1))
            txt_pool = ec(tc.tile_pool(name="txt", bufs=1))
            wqs_pool = ec(tc.tile_pool(name="wqs", bufs=2))
            vt_pool = ec(tc.tile_pool(name="vt", bufs=1))
            ktg_pool = ec(tc.tile_pool(name="ktg", bufs=2))
            masktl_pool = ec(tc.tile_pool(name="masktl", bufs=1))
            masklt_pool = ec(tc.tile_pool(name="masklt", bufs=1))
            qv_pool = ec(tc.tile_pool(name="qv", bufs=1))
            p_pool = ec(tc.tile_pool(name="p", bufs=2))
            pt_pool = ec(tc.tile_pool(name="pt", bufs=2))
            rvb_pool = ec(tc.tile_pool(name="rvb", bufs=2))
            small_pool = ec(tc.tile_pool(name="small", bufs=4))
            ostage_pool = ec(tc.tile_pool(name="ostage", bufs=1))
            ov_pool = ec(tc.tile_pool(name="ovstage", bufs=1))
            ps_bank = ec(tc.tile_pool(name="ps_bank", bufs=2, space="PSUM"))  # 2-bank
            ps_acc = ec(tc.tile_pool(name="ps_acc", bufs=4, space="PSUM"))  # 1-bank

            # ---------------- constants ----------------
            wvv_sb = consts.tile([128, kt, A], BF16)
            nc.gpsimd.dma_start(out=wvv_sb, in_=wvv.rearrange("(k p) a -> p k a", p=128))
            wk_sb = consts.tile([128, kv, A], BF16)
            nc.gpsimd.dma_start(out=wk_sb, in_=wk.rearrange("(k p) a -> p k a", p=128))
            wvt_sb = consts.tile([128, kv, A], BF16)
            nc.gpsimd.dma_start(out=wvt_sb, in_=wvt.rearrange("(k p) a -> p k a", p=128))

            bk_sb = consts.tile([128, G], F32)
            nc.sync.dma_start(out=bk_sb, in_=bk.rearrange("(g p) -> p g", p=128))
            # bq as a single-partition row for the K=1 bias matmul
            bq_row = consts.tile([1, A], F32)
            nc.sync.dma_start(out=bq_row, in_=bq[None, :])
            ones_row = consts.tile([1, L], F32)
            nc.vector.memset(ones_row, 1.0)

            # bvt/SCALE and bvv/SCALE broadcast along partitions
            bvt_sb = consts.tile([128, A], BF16)
            nc.gpsimd.dma_start(out=bvt_sb, in_=bcast_ap(bvt))
            nc.vector.tensor_scalar_mul(bvt_sb, bvt_sb, 1.0 / SCALE)
            bvv_sb = consts.tile([128, A], BF16)
            nc.gpsimd.dma_start(out=bvv_sb, in_=bcast_ap(bvv))
            nc.vector.tensor_scalar_mul(bvv_sb, bvv_sb, 1.0 / SCALE)

            identity = consts.tile([128, 128], F32)
            make_identity(nc, identity)

            neg_c = consts.tile([128, 1], F32)
            nc.vector.memset(neg_c, -EXP_C)

            for bi in range(b_pc):
                # ---------------- per-batch loads ----------------
                video_sb = vid_pool.tile([128, kv, t], BF16, tag="video")
                nc.gpsimd.dma_start(
                    out=video_sb, in_=videofea[bi].rearrange("(k p) t -> p k t", p=128)
                )
                text_sb = txt_pool.tile([128, TD], F32, tag="text")
                nc.sync.dma_start(out=text_sb, in_=textfea[bi])

                # mask: int32 [T, L] --cast--> bf16, bounce through DRAM for the
                # xbar transpose into [L, T]
                m_tl = masktl_pool.tile([128, nt, L], BF16, tag="mtl")
                nc.gpsimd.dma_start(
                    out=m_tl, in_=mask[bi].rearrange("(tt p) l -> p tt l", p=128)
                )
                nc.sync.dma_start(
                    out=m_scr[bi].rearrange("(tt p) l -> p tt l", p=128), in_=m_tl
                )
                m_lt = masklt_pool.tile([L, t], BF16, tag="mlt")
                nc.sync.dma_start_transpose(out=m_lt, in_=m_scr[bi])

                # textfea^T via PE transpose: f32 feeds q, bf16 feeds vv
                textT_sb = txt_pool.tile([128, kt, L], F32, tag="textT")
                textT_bf = txt_pool.tile([128, kt, L], BF16, tag="textT_bf")
                for k in range(kt):
                    ps_tr = ps_acc.tile([128, 128], F32, tag="acc")
                    nc.tensor.transpose(ps_tr, text_sb[:, bass.ts(k, 128)], identity)
                    nc.vector.tensor_copy(out=textT_sb[:, k], in_=ps_tr)
                    nc.vector.tensor_copy(out=textT_bf[:, k], in_=ps_tr)

                # ---------------- projections ----------------
                # qT [A(g), L] f32; Wq streamed from DRAM per k-tile (k-outer);
                # one [128, 8*L] psum tile held across the k loop; bq added via
                # a K=1 matmul against a ones row. Only the very first matmul
                # sets start (it clears the whole bank's has_written bits);
                # later first-touches overwrite per element.
                ps_q = ps_bank.tile([128, G, L], F32, tag="bank", name="ps_q")
                nc.vector.memset(ps_q, 0.0)
                for k in range(kt):
                    wq_k = wqs_pool.tile([128, A], F32, tag="wq")
                    nc.sync.dma_start(out=wq_k, in_=wq[bass.ts(k, 128), :])
                    for g in range(G):
                        nc.tensor.matmul(
                            ps_q[:, g],
                            lhsT=wq_k[:, bass.ts(g, 128)],
                            rhs=textT_sb[:, k],
                            start=False,
                            stop=False,
                            skip_group_check=True,
                        )
                qT_sb = qv_pool.tile([128, G, L], F32, tag="qT")
                for g in range(G):
                    nc.tensor.matmul(
                        ps_q[:, g],
                        lhsT=bq_row[:, bass.ts(g, 128)],
                        rhs=ones_row,
                        start=False,
                        stop=(g == G - 1),
                        skip_group_check=True,
                    )
                nc.vector.tensor_copy(out=qT_sb, in_=ps_q)

                # vv [L, A] bf16, pre-scaled by 1/32, bvv/32 added
                vv_sb = qv_pool.tile([128, A], BF16, tag="vv")
                for half in range(2):
                    ps = ps_acc.tile([128, 512], F32, tag="acc")
                    for k in range(kt):
                        nc.tensor.matmul(
                            ps,
                            lhsT=textT_bf[:, k],
                            rhs=wvv_sb[:, k, bass.ts(half, 512)],
                            start=(k == 0),
                            stop=(k == kt - 1),
                        )
                    nc.vector.scalar_tensor_tensor(
                        out=vv_sb[:, bass.ts(half, 512)],
                        in0=ps,
                        scalar=1.0 / SCALE,
                        op0=AluOpType.mult,
                        in1=bvv_sb[:, bass.ts(half, 512)],
                        op1=AluOpType.add,
                    )

                # vt [T, A] bf16 resident, pre-scaled by 1/32
                vt_sb = vt_pool.tile([128, nt, A], BF16, tag="vt")
                for mt in range(nt):
                    ps = ps_bank.tile([128, A], F32, tag="bank")
                    for k in range(kv):
                        for half in range(2):
                            nc.tensor.matmul(
                                ps[:, bass.ts(half, 512)],
                                lhsT=video_sb[:, k, bass.ts(mt, 128)],
                                rhs=wvt_sb[:, k, bass.ts(half, 512)],
                                start=(k == 0),
                                stop=(k == kv - 1),
                            )
                    nc.vector.tensor_scalar(
                        out=vt_sb[:, mt], in0=ps, scalar1=1.0 / SCALE,
                        scalar2=None, op0=AluOpType.mult,
                    )

                # -------- per-group attention (skew-1 software pipeline) -----
                state = {}

                def phase_a(g):
                    st = {}
                    # kT_g [d, T] f32 recomputed from resident video (no DRAM
                    # spill: saves 64MB/core of queue traffic + read latency)
                    kT_g = ktg_pool.tile([128, t], F32, tag="kT_g", name="kT_g")
                    for pair in range(npair):
                        ps = ps_bank.tile([128, 1024], F32, tag="bank", name="ps_kt")
                        for k in range(kv):
                            for ch in range(2):
                                nc.tensor.matmul(
                                    ps[:, bass.ts(ch, 512)],
                                    lhsT=wk_sb[:, k, bass.ts(g, 128)],
                                    rhs=video_sb[:, k, bass.ds(pair * 1024 + ch * 512, 512)],
                                    start=(k == 0),
                                    stop=(k == kv - 1),
                                )
                        nc.vector.tensor_scalar(
                            out=kT_g[:, bass.ts(pair, 1024)], in0=ps,
                            scalar1=bk_sb[:, g : g + 1], scalar2=None,
                            op0=AluOpType.add,
                        )

                    # S [L, T] f32; exp chunks; one wide mask-mult
                    p_sb = p_pool.tile([L, t], BF16, tag="P", name="p_sb")
                    for ch in range(nch):
                        ps = ps_acc.tile([128, 512], F32, tag="acc", name="ps_s")
                        nc.tensor.matmul(
                            ps, lhsT=qT_sb[:, g], rhs=kT_g[:, bass.ts(ch, 512)],
                            start=True, stop=True,
                        )
                        nc.scalar.activation(
                            out=p_sb[:, bass.ts(ch, 512)], in_=ps, func=AF.Exp,
                            bias=neg_c,
                        )
                    nc.vector.tensor_tensor(
                        out=p_sb, in0=p_sb, in1=m_lt, op=AluOpType.mult
                    )
                    rs = small_pool.tile([128, 1], F32, tag="rs", name="rs")
                    nc.vector.tensor_reduce(
                        out=rs, in_=p_sb, axis=mybir.AxisListType.X, op=AluOpType.add
                    )
                    recip_t = small_pool.tile([128, 1], F32, tag="recip_t", name="rt")
                    nc.vector.reciprocal(out=recip_t, in_=rs)

                    # S^T [T, L] tiles; exp + mask per 4-tile chunk; colsums
                    pt_sb = pt_pool.tile([128, nt, L], BF16, tag="PT", name="pt_sb")
                    cs = small_pool.tile([128, nt], F32, tag="cs", name="cs")
                    for grp in range(nch):
                        ps = ps_acc.tile([128, 4, 128], F32, tag="acc", name="ps_st")
                        for j in range(4):
                            tt = grp * 4 + j
                            nc.tensor.matmul(
                                ps[:, j],
                                lhsT=kT_g[:, bass.ts(tt, 128)],
                                rhs=qT_sb[:, g],
                                start=(j == 0),
                                stop=(j == 3),
                                skip_group_check=True,
                            )
                        sl = slice(grp * 4, grp * 4 + 4)
                        nc.scalar.activation(
                            out=pt_sb[:, sl], in_=ps, func=AF.Exp, bias=neg_c
                        )
                        nc.vector.tensor_tensor(
                            out=pt_sb[:, sl], in0=pt_sb[:, sl], in1=m_tl[:, sl],
                            op=AluOpType.mult,
                        )
                        nc.vector.tensor_reduce(
                            out=cs[:, sl], in_=pt_sb[:, sl],
                            axis=mybir.AxisListType.X, op=AluOpType.add,
                        )
                    rv = small_pool.tile([128, nt], F32, tag="rv", name="rv")
                    nc.vector.reciprocal(out=rv, in_=cs)
                    nc.sync.dma_start(
                        out=rv_scr[g].rearrange("(tt p) -> p tt", p=128), in_=rv
                    )
                    rvb = rvb_pool.tile([128, t], F32, tag="rvb", name="rvb")
                    nc.sync.dma_start(out=rvb, in_=bcast_ap(rv_scr[g]))

                    st["p_sb"], st["pt_sb"] = p_sb, pt_sb
                    st["recip_t"], st["rvb"] = recip_t, rvb
                    state[g] = st

                def phase_b(g):
                    st = state.pop(g)
                    p_sb, pt_sb = st["p_sb"], st["pt_sb"]

                    # out_t[L, d_g] = (PT.T @ vt_g) * recip_t + bvt_g/32
                    ps_ot = ps_acc.tile([128, 128], F32, tag="acc", name="ps_ot")
                    for tt in range(nt):
                        nc.tensor.matmul(
                            ps_ot,
                            lhsT=pt_sb[:, tt],
                            rhs=vt_sb[:, tt, bass.ts(g, 128)],
                            start=(tt == 0),
                            stop=(tt == nt - 1),
                        )
                    ot = ostage_pool.tile([128, 128], F32, tag="ot", name="ot")
                    nc.vector.scalar_tensor_tensor(
                        out=ot,
                        in0=ps_ot,
                        scalar=st["recip_t"],
                        op0=AluOpType.mult,
                        in1=bvt_sb[:, bass.ts(g, 128)],
                        op1=AluOpType.add,
                    )
                    nc.sync.dma_start(out=out_t[bi, :, bass.ts(g, 128)], in_=ot)

                    # out_v^T[d_g, T] = vv_g.T @ P ; column normalizer applied
                    # at the PSUM->SBUF copy via the broadcast reciprocal
                    ovT = ov_pool.tile([128, t], F32, tag="ovT", name="ovT")
                    for pair in range(npair):
                        ps = ps_bank.tile([128, 1024], F32, tag="bank", name="ps_ov")
                        for ch in range(2):
                            nc.tensor.matmul(
                                ps[:, bass.ts(ch, 512)],
                                lhsT=vv_sb[:, bass.ts(g, 128)],
                                rhs=p_sb[:, bass.ds(pair * 1024 + ch * 512, 512)],
                                start=True,
                                stop=True,
                            )
                        nc.vector.tensor_tensor(
                            out=ovT[:, bass.ts(pair, 1024)],
                            in0=ps,
                            in1=st["rvb"][:, bass.ts(pair, 1024)],
                            op=AluOpType.mult,
                        )
                    nc.sync.dma_start(out=out_v[bi, bass.ts(g, 128), :], in_=ovT)

                for g in range(G):
                    phase_a(g)
                    if g >= 1:
                        phase_b(g - 1)
                phase_b(G - 1)

    nc.compile()
    return nc


_NC_CACHE: dict = {}


def _get_nc():
    if "nc" not in _NC_CACHE:
        _NC_CACHE["nc"] = build_kernel()
    return _NC_CACHE["nc"]


def kernel(**inputs) -> tuple:
    nc = _get_nc()
    in_maps = []
    for c in range(N_CORES):
        sl = slice(c * B_PC, (c + 1) * B_PC)
        in_maps.append(
            {
                "videofea": np.ascontiguousarray(inputs["videofea"][sl]),
                "textfea": np.ascontiguousarray(inputs["textfea"][sl]),
                "mask": np.ascontiguousarray(inputs["mask"][sl]),
                "Wq": np.asarray(inputs["Wq"]),
                "bq": np.asarray(inputs["bq"]),
                "Wk": np.asarray(inputs["Wk"]),
                "bk": np.asarray(inputs["bk"]),
                "Wvv": np.asarray(inputs["Wvv"]),
                "bvv": np.asarray(inputs["bvv"]),
                "Wvt": np.asarray(inputs["Wvt"]),
                "bvt": np.asarray(inputs["bvt"]),
            }
        )
    res = run_bass_kernel_spmd(nc, in_maps, core_ids=list(range(N_CORES)))
    out_v = np.concatenate([r["out_v"] for r in res.results], axis=0)  # [B, A, T]
    out_t = np.concatenate([r["out_t"] for r in res.results], axis=0)  # [B, L, A]
    return out_v, out_t
